# revision 33
# baseline (speedup 1.0000x reference)
"""Trainium2 Bass kernel for the CRF negative-log-likelihood (B=128, S=2048, C=128).

Distribution: data-parallel over batch, 16 sequences per NeuronCore (8 cores).

Per-core algorithm (all in "probability space" to turn the logsumexp scan into
matmuls on the PE):

  p_t = (E^T p_{t-1}) * exp(emit_t - C0),   E = exp(transitions)

The 2048-step scan is split into NCH=32 time-chunks of L=64 steps, processed
simultaneously as a 512-wide batch (chunk x b) so every instruction is wide
and per-op fixed costs amortize; the batch is further split into two 256-wide
half-chains so PE matmuls and DVE multiplies of the two halves interleave.
Pass 1 runs every chunk from a neutral start; a 32-step fixup pass re-runs
each chunk's head seeded with pass-1's previous-chunk final state.  The fast
mixing of the positive transfer operator makes this exact to ~1e-7 (validated
in numpy).  Renormalizations every 32 steps (divide by column sums, tracked
in log-offset accumulators) keep everything in f32 range.

The gold path score is computed with one-hot algebra on the same engines:
  onehot = (tags_rep == iota)            (tags pre-replicated across partitions)
  U      = T^T @ onehot_prev + emit      (PE, PSUM-accumulated)
  gold  += colsum(onehot * U)            (ones-column matmul into a PSUM acc)

Final per-core output: loss_partial[16] = gold_b - logZ_b; host returns
-mean over all 128.
"""

import sys

sys.path.insert(0, "/opt/trn_rl_repo")

from contextlib import ExitStack

import numpy as np

import concourse.bass as bass
import concourse.bacc as bacc_mod
import concourse.mybir as mybir
import concourse.tile as tile
from concourse.masks import make_identity

dt = mybir.dt
Alu = mybir.AluOpType
Act = mybir.ActivationFunctionType

B, S, C = 128, 2048, 128
NCORES = 8
BL = B // NCORES          # 16 sequences per core
NCH = 32                  # time chunks
L = S // NCH              # 64 macro-steps
W = NCH * BL              # 512 working columns
K_RENORM = 32
F = 32                    # fixup pass length
C0 = 5.8                  # per-step growth bias folded into exp(emit - C0)

TB = 1                    # macro-steps per gold/stream block
W2 = (NCH // 2) * BL      # half-chain width (G=2 scan interleave)
NBLK = L // TB            # 64 blocks
WB = TB * W               # 512 columns per block
GOLD_SEG = 16             # blocks per gold PSUM accumulation segment

f32 = dt.float32
bf16 = dt.bfloat16
u8 = dt.uint8

DBG_NO_GOLD = False
DBG_NO_RENORM = False
DBG_NO_PASS2 = False
DBG_NBLK = NBLK
DBG_GOLD_LVL = 3


def build_program() -> bass.Bass:
    nc = bacc_mod.Bacc()

    emt = nc.declare_dram_parameter("emt", [C, L * W], f32, isOutput=False)
    tagsr = nc.declare_dram_parameter("tagsr", [C, L * W], u8, isOutput=False)
    trans = nc.declare_dram_parameter("trans", [C, C], f32, isOutput=False)
    stend = nc.declare_dram_parameter("stend", [C, 2], f32, isOutput=False)
    out = nc.declare_dram_parameter("out", [BL], f32, isOutput=True)

    do_gold = not DBG_NO_GOLD
    full = DBG_NBLK == NBLK

    with tile.TileContext(nc) as tc, ExitStack() as ctx:
        singles = ctx.enter_context(tc.tile_pool(name="singles", bufs=1))
        empool = ctx.enter_context(tc.tile_pool(name="empool", bufs=3))
        embf_pool = ctx.enter_context(tc.tile_pool(name="embf", bufs=2))
        tgpool = ctx.enter_context(tc.tile_pool(name="tgpool", bufs=2))
        ohpool = ctx.enter_context(tc.tile_pool(name="ohpool", bufs=3))
        gupool = ctx.enter_context(tc.tile_pool(name="gupool", bufs=2))
        xpool = ctx.enter_context(tc.tile_pool(name="xpool", bufs=6))
        tmp_pool = ctx.enter_context(tc.tile_pool(name="tmp", bufs=5))
        s_psum = ctx.enter_context(tc.tile_pool(name="s_psum", bufs=3, space="PSUM"))
        u_psum = ctx.enter_context(tc.tile_pool(name="u_psum", bufs=2, space="PSUM"))
        c_psum = ctx.enter_context(tc.tile_pool(name="c_psum", bufs=1, space="PSUM"))
        g_psum = ctx.enter_context(tc.tile_pool(name="g_psum", bufs=1, space="PSUM"))
        r_psum = ctx.enter_context(tc.tile_pool(name="r_psum", bufs=1, space="PSUM"))

        # ---- constants -------------------------------------------------
        t_sb = singles.tile([C, C], f32)
        nc.sync.dma_start(out=t_sb, in_=trans[:, :])
        stend_sb = singles.tile([C, 2], f32)
        nc.sync.dma_start(out=stend_sb, in_=stend[:, :])
        start_sb = stend_sb[:, 0:1]
        end_sb = stend_sb[:, 1:2]

        # E (bf16) for the scan; T (bf16) for the gold transition lookup
        e_bf = singles.tile([C, C], bf16)
        nc.scalar.activation(e_bf, t_sb, Act.Exp)
        t_bf = singles.tile([C, C], bf16)
        nc.vector.tensor_copy(t_bf, t_sb)
        # u = exp(end)
        u_sb = singles.tile([C, 1], f32)
        nc.scalar.activation(u_sb, end_sb, Act.Exp)

        ident_bf = singles.tile([C, C], bf16)
        make_identity(nc, ident_bf)
        onescol_bf = singles.tile([C, 1], bf16)
        nc.vector.memset(onescol_bf, 1.0)
        ones128_bf = singles.tile([C, C], bf16)
        nc.vector.memset(ones128_bf, 1.0)

        # -C0 bias produced on ACT so the emission exp carries only the
        # DMA wait (big ISA structs have a single sync-wait slot)
        negc0_sb = singles.tile([C, 1], f32)
        nc.scalar.activation(negc0_sb, stend_sb[:, 0:1], Act.Copy, bias=-C0, scale=0.0)
        # early DVE read of stend_sb so later DVE ops that use start/end
        # slices carry no extra DMA-sem wait (some ISA structs allow only
        # one sync wait)
        stend_dve = singles.tile([C, 2], f32)
        nc.vector.tensor_copy(stend_dve, stend_sb)
        expstart_sb = singles.tile([C, 1], f32)
        nc.scalar.activation(expstart_sb, stend_sb[:, 0:1], Act.Exp)

        iota_i = singles.tile([C, 1], dt.int32)
        nc.gpsimd.iota(iota_i, pattern=[[0, 1]], base=0, channel_multiplier=1)
        iota_f = singles.tile([C, 1], f32)
        nc.vector.tensor_copy(iota_f, iota_i)

        # resident exp(emit - C0) for the whole sequence (bf16, 64KB/partition)
        ee_all = singles.tile([C, L * W], bf16)
        # log-offset accumulator (renorm logs from both passes), [1, W]
        o_acc = singles.tile([1, W], f32)
        nc.vector.memset(o_acc, 0.0)
        f1_sb = singles.tile([C, W], bf16)
        oh_zero = singles.tile([C, W], bf16)
        nc.vector.memset(oh_zero, 0.0)
        gold_sb = singles.tile([1, WB], f32)
        nc.vector.memset(gold_sb, 0.0)

        gold_acc = g_psum.tile([1, WB], f32, tag="gold_acc", name="gold_acc") if (do_gold and DBG_GOLD_LVL >= 3) else None

        # PE warm-up matmuls: absorb the ACT tick, then the GPSIMD tick, so
        # every later matmul waits on at most one new proc (big ISA structs
        # have a single sync-wait slot)
        warm_ps = s_psum.tile([1, 4], f32, tag="s", name="warm")
        nc.tensor.matmul(warm_ps, lhsT=e_bf[:, 0:1], rhs=e_bf[:, 0:4], start=True, stop=True)
        warm2_ps = s_psum.tile([1, 4], f32, tag="s", name="warm2")
        nc.tensor.matmul(warm2_ps, lhsT=ident_bf[:, 0:1], rhs=ident_bf[:, 0:4], start=True, stop=True)

        # ---------------------------------------------------------------
        def renorm(x_new, track, h):
            crep = c_psum.tile([C, W2], f32)
            nc.tensor.matmul(crep, lhsT=ones128_bf, rhs=x_new, start=True, stop=True)
            rec = tmp_pool.tile([C, W2], f32, tag="rec")
            nc.vector.reciprocal(rec, crep)
            x_n = xpool.tile([C, W2], bf16, tag=f"X{h}", name=f"xn{h}")
            nc.vector.tensor_tensor(x_n, x_new, rec, op=Alu.mult)
            if track:
                lg = tmp_pool.tile([1, W2], f32, tag="lg")
                nc.scalar.activation(lg, crep[0:1, :], Act.Ln)
                nc.vector.tensor_tensor(
                    o_acc[0:1, h * W2 : (h + 1) * W2],
                    o_acc[0:1, h * W2 : (h + 1) * W2], lg, op=Alu.add,
                )
            return x_n

        def scan_step_half(x, t, h, is_pass1):
            # half h covers columns [h*W2, (h+1)*W2) of the chunk x batch grid
            s_ps = s_psum.tile([C, W2], f32, tag="s", name=f"s{h}")
            nc.tensor.matmul(s_ps, lhsT=e_bf, rhs=x, start=True, stop=True)
            x_new = xpool.tile([C, W2], bf16, tag=f"X{h}", name=f"x{h}")
            base = t * W + h * W2
            nc.vector.tensor_tensor(
                x_new, s_ps, ee_all[:, base : base + W2], op=Alu.mult
            )
            if t == 0 and h == 0:
                # true chunk-0 init: exp(start + emit_0 - C0) == ee cols
                nc.vector.tensor_copy(x_new[:, 0:BL], ee_all[:, 0:BL])
            x = x_new
            if (t + 1) % K_RENORM == 0 and not DBG_NO_RENORM:
                if is_pass1:
                    if t != L - 1:
                        x = renorm(x, track=(t >= F), h=h)
                else:
                    x = renorm(x, track=True, h=h)
            return x

        # ---- pass 1 (gold interleaved) --------------------------------
        xa = xpool.tile([C, W2], bf16, tag="X0", name="xa0")
        nc.vector.memset(xa, 1.0)
        xb = xpool.tile([C, W2], bf16, tag="X1", name="xb0")
        nc.vector.memset(xb, 1.0)

        oh_prev_half = oh_zero[:, :]        # one-hot of previous macro-step
        keep_tg0 = None
        last_oh = None

        for blk in range(DBG_NBLK):
            t0 = TB * blk
            col0 = t0 * W

            em_t = empool.tile([C, WB], f32, tag="em")
            nc.sync.dma_start(out=em_t, in_=emt[:, col0 : col0 + WB])
            # resident scaled-exp emissions
            nc.scalar.activation(
                ee_all[:, col0 : col0 + WB], em_t, Act.Exp, bias=negc0_sb
            )
            if blk == 0:
                # chunk-0 init cols get exp(start + emit_0 - C0): scale the
                # just-computed exp(emit-C0) by exp(start); all operands are
                # ACT-local so this op needs no cross-engine sync wait
                nc.scalar.activation(
                    ee_all[:, 0:BL], ee_all[:, 0:BL], Act.Copy,
                    scale=expstart_sb,
                )

            if do_gold:
                # bf16 emissions for the gold PSUM accumulation
                em_bf = embf_pool.tile([C, WB], bf16, tag="embf")
                nc.vector.tensor_copy(em_bf, em_t)
                # gold: tags one-hot for this block (built on the otherwise
                # idle GPSIMD engine)
                tgrep = tgpool.tile([C, WB], u8, tag="tg")
                nc.sync.dma_start(out=tgrep, in_=tagsr[:, col0 : col0 + WB])
                oh = ohpool.tile([C, WB], bf16, tag="oh")
                nc.gpsimd.tensor_scalar(oh, tgrep, iota_f, None, op0=Alu.is_equal)
                if blk == 0:
                    keep_tg0 = tgpool.tile([C, W], u8, tag="tg0")
                    nc.gpsimd.tensor_copy(keep_tg0, tgrep[:, 0:W])

                if DBG_GOLD_LVL >= 2:
                    # U = T^T @ onehot_prev (+ emit)
                    u_ps = u_psum.tile([C, WB], f32, tag="U")
                    nc.tensor.matmul(
                        u_ps, lhsT=t_bf, rhs=oh_prev_half,
                        start=True, stop=False, skip_group_check=True,
                    )
                    nc.tensor.matmul(
                        u_ps, lhsT=ident_bf, rhs=em_bf, start=False, stop=True,
                        skip_group_check=True,
                    )
                    # drain U+em to SBUF on ACT, select on GPSIMD: keeps the
                    # serial-scan engine (DVE) free
                    usel_sb = gupool.tile([C, WB], bf16, tag="usb")
                    nc.scalar.activation(usel_sb, u_ps, Act.Copy)
                    gu = gupool.tile([C, WB], bf16, tag="gu")
                    nc.gpsimd.tensor_tensor(gu, oh, usel_sb, op=Alu.mult)
                    if DBG_GOLD_LVL >= 3:
                        seg_first = blk % GOLD_SEG == 0
                        seg_last = blk % GOLD_SEG == GOLD_SEG - 1
                        nc.tensor.matmul(
                            gold_acc, lhsT=onescol_bf, rhs=gu, start=seg_first,
                            stop=seg_last, skip_group_check=True,
                        )
                        if seg_last:
                            nc.vector.tensor_tensor(
                                gold_sb, gold_sb, gold_acc, op=Alu.add
                            )
                oh_prev_half = oh[:, :]
                last_oh = oh

            # scan macro-steps for this block
            for ts in range(TB):
                xa = scan_step_half(xa, t0 + ts, 0, is_pass1=True)
                xb = scan_step_half(xb, t0 + ts, 1, is_pass1=True)

        nc.vector.tensor_copy(f1_sb[:, 0:W2], xa)
        nc.vector.tensor_copy(f1_sb[:, W2:W], xb)

        if do_gold and full and DBG_GOLD_LVL >= 3:
            # deferred chunk-boundary gold terms: T[tag(L-1,c-1), tag(0,c)]
            u0_ps = u_psum.tile([C, (NCH - 1) * BL], f32, tag="U")
            nc.tensor.matmul(
                u0_ps, lhsT=t_bf, rhs=last_oh[:, 0 : (NCH - 1) * BL],
                start=True, stop=True, skip_group_check=True,
            )
            oh0 = ohpool.tile([C, (NCH - 1) * BL], bf16, tag="oh0")
            nc.vector.tensor_scalar(
                oh0, keep_tg0[:, BL:W], iota_f, None, op0=Alu.is_equal
            )
            gu0 = gupool.tile([C, (NCH - 1) * BL], bf16, tag="gu")
            nc.vector.tensor_tensor(gu0, oh0, u0_ps, op=Alu.mult)
            nc.tensor.matmul(
                gold_acc[0:1, BL:W], lhsT=onescol_bf, rhs=gu0,
                start=True, stop=False, skip_group_check=True,
            )
            # start_transitions term: onehot(tag_0, chunk 0) * start
            oh_s = ohpool.tile([C, BL], bf16, tag="oh0")
            nc.vector.tensor_scalar(
                oh_s, keep_tg0[:, 0:BL], iota_f, None, op0=Alu.is_equal
            )
            gstart = gupool.tile([C, BL], bf16, tag="gu")
            nc.vector.tensor_scalar(
                gstart, oh_s, stend_dve[:, 0:1], None, op0=Alu.mult
            )
            nc.tensor.matmul(
                gold_acc[0:1, 0:BL], lhsT=onescol_bf, rhs=gstart,
                start=False, stop=False, skip_group_check=True,
            )
            # end_transitions term: onehot(tag_last) * end
            gend = gupool.tile([C, BL], bf16, tag="gu")
            nc.vector.tensor_scalar(
                gend, last_oh[:, WB - BL : WB], stend_dve[:, 1:2], None, op0=Alu.mult
            )
            nc.tensor.matmul(
                gold_acc[0:1, WB - BL : WB], lhsT=onescol_bf, rhs=gend,
                start=False, stop=True, skip_group_check=True,
            )
            # the deferred segment's first matmul used start=True, zeroing the
            # whole bank, and every column is written exactly once -> one add
            nc.vector.tensor_tensor(
                gold_sb[0:1, 0:WB], gold_sb[0:1, 0:WB], gold_acc[0:1, 0:WB],
                op=Alu.add,
            )

        # ---- pass 2: 32-step head fixup -------------------------------
        if not DBG_NO_PASS2:
            x2a = xpool.tile([C, W2], bf16, tag="X0", name="x2a")
            nc.vector.memset(x2a, 1.0)
            nc.vector.tensor_copy(x2a[:, BL:W2], f1_sb[:, 0 : W2 - BL])
            x2b = xpool.tile([C, W2], bf16, tag="X1", name="x2b")
            nc.vector.tensor_copy(x2b, f1_sb[:, W2 - BL : W - BL])
            xa, xb = x2a, x2b
            for t in range(F):
                xa = scan_step_half(xa, t, 0, is_pass1=False)
                xb = scan_step_half(xb, t, 1, is_pass1=False)

        # ---- final assembly -------------------------------------------
        # logZ_b = sum_c o_acc + 2048*C0 + log(u^T F1[last chunk])
        v16 = tmp_pool.tile([C, BL], bf16, tag="v16")
        nc.vector.tensor_scalar(
            v16, f1_sb[:, W - BL : W], u_sb, None, op0=Alu.mult
        )
        ud_ps = r_psum.tile([1, BL], f32)
        nc.tensor.matmul(ud_ps, lhsT=onescol_bf, rhs=v16, start=True, stop=True)
        logud = tmp_pool.tile([1, BL], f32, tag="asm")
        nc.scalar.activation(logud, ud_ps, Act.Ln)

        # sum o_acc over chunks: view [1, (c b)] -> [1, b, c], reduce inner
        oz = tmp_pool.tile([1, BL], f32, tag="asm")
        nc.vector.tensor_reduce(
            oz,
            o_acc.rearrange("p (c b) -> p b c", c=NCH, b=BL),
            axis=mybir.AxisListType.X,
            op=Alu.add,
        )
        loss = tmp_pool.tile([1, BL], f32, tag="asm")
        if do_gold and full and DBG_GOLD_LVL >= 3:
            # gold: view [1, (ts c b)] -> [1, b, (ts c)], reduce inner
            gr = tmp_pool.tile([1, BL], f32, tag="asm")
            nc.vector.tensor_reduce(
                gr,
                gold_sb.rearrange("p (t c b) -> p b (t c)", t=TB, c=NCH, b=BL),
                axis=mybir.AxisListType.X,
                op=Alu.add,
            )
            nc.vector.tensor_tensor(loss, gr, oz, op=Alu.subtract)
        else:
            nc.vector.tensor_scalar(loss, oz, -1.0, None, op0=Alu.mult)
        nc.vector.tensor_tensor(loss, loss, logud, op=Alu.subtract)
        nc.vector.tensor_scalar(loss, loss, -float(S) * C0, None, op0=Alu.add)
        nc.sync.dma_start(out=out[:], in_=loss[0:1, :])

    nc.finalize()
    return nc


_PROGRAM = None


def _get_program():
    global _PROGRAM
    if _PROGRAM is None:
        _PROGRAM = build_program()
    return _PROGRAM


def make_in_maps(emissions, transitions, start_transitions, end_transitions, tags):
    emissions = np.asarray(emissions, np.float32)
    transitions = np.asarray(transitions, np.float32)
    start_transitions = np.asarray(start_transitions, np.float32)
    end_transitions = np.asarray(end_transitions, np.float32)
    tags = np.asarray(tags)

    stend = np.ascontiguousarray(
        np.stack([start_transitions, end_transitions], axis=1)
    ).astype(np.float32)

    in_maps = []
    for k in range(NCORES):
        sl = slice(k * BL, (k + 1) * BL)
        # [BL, S, C] -> [C, S, BL] -> [C, NCH, L, BL] -> [C, L, NCH, BL]
        em = emissions[sl].transpose(2, 1, 0).reshape(C, NCH, L, BL)
        em = np.ascontiguousarray(em.transpose(0, 2, 1, 3)).reshape(C, L * W)
        # tags -> [L, NCH*BL] u8, replicated across 128 partitions
        tg = tags[sl].T.reshape(NCH, L, BL).transpose(1, 0, 2).reshape(L * W)
        tg = np.ascontiguousarray(
            np.broadcast_to(tg.astype(np.uint8)[None, :], (C, L * W))
        )
        in_maps.append(
            {"emt": em, "tagsr": tg, "trans": transitions, "stend": stend}
        )
    return in_maps


def kernel(emissions, transitions, start_transitions, end_transitions, tags, mask):
    from concourse.bass_utils import run_bass_kernel_spmd

    nc = _get_program()
    in_maps = make_in_maps(
        emissions, transitions, start_transitions, end_transitions, tags
    )
    res = run_bass_kernel_spmd(nc, in_maps, list(range(NCORES))).results
    parts = np.concatenate([np.asarray(r["out"], np.float32) for r in res])
    return np.float32(-parts.mean())


# revision 36
# speedup vs baseline: 1.0489x; 1.0489x over previous
"""Trainium2 Bass kernel for the CRF negative-log-likelihood (B=128, S=2048, C=128).

Distribution: data-parallel over batch, 16 sequences per NeuronCore (8 cores).

Per-core algorithm (all in "probability space" to turn the logsumexp scan into
matmuls on the PE):

  p_t = (E^T p_{t-1}) * exp(emit_t - C0),   E = exp(transitions)

The 2048-step scan is split into NCH=32 time-chunks of L=64 steps, processed
simultaneously as a 512-wide batch (chunk x b) so every instruction is wide
and per-op fixed costs amortize; the batch is further split into two 256-wide
half-chains so PE matmuls and DVE multiplies of the two halves interleave.
Pass 1 runs every chunk from a neutral start; a 32-step fixup pass re-runs
each chunk's head seeded with pass-1's previous-chunk final state.  The fast
mixing of the positive transfer operator makes this exact to ~1e-7 (validated
in numpy).  Renormalizations every 32 steps (divide by column sums, tracked
in log-offset accumulators) keep everything in f32 range.

The gold path score is computed with one-hot algebra on the same engines:
  onehot = (tags_rep == iota)            (tags pre-replicated across partitions)
  U      = T^T @ onehot_prev + emit      (PE, PSUM-accumulated)
  gold  += colsum(onehot * U)            (ones-column matmul into a PSUM acc)

Final per-core output: loss_partial[16] = gold_b - logZ_b; host returns
-mean over all 128.
"""

import sys

sys.path.insert(0, "/opt/trn_rl_repo")

from contextlib import ExitStack

import numpy as np

import concourse.bass as bass
import concourse.bacc as bacc_mod
import concourse.mybir as mybir
import concourse.tile as tile
from concourse.masks import make_identity

dt = mybir.dt
Alu = mybir.AluOpType
Act = mybir.ActivationFunctionType

B, S, C = 128, 2048, 128
NCORES = 8
BL = B // NCORES          # 16 sequences per core
NCH = 32                  # time chunks
L = S // NCH              # 64 macro-steps
W = NCH * BL              # 512 working columns
K_RENORM = 32
F = 32                    # fixup pass length
C0 = 5.8                  # per-step growth bias folded into exp(emit - C0)

TB = 1                    # macro-steps per gold/stream block
W2 = (NCH // 2) * BL      # half-chain width (G=2 scan interleave)
NBLK = L // TB            # 64 blocks
WB = TB * W               # 512 columns per block
GOLD_SEG = 16             # blocks per gold PSUM accumulation segment

f32 = dt.float32
bf16 = dt.bfloat16
u8 = dt.uint8

DBG_NO_GOLD = False
DBG_NO_RENORM = False
DBG_NO_PASS2 = False
DBG_NBLK = NBLK
DBG_GOLD_LVL = 3


def build_program() -> bass.Bass:
    nc = bacc_mod.Bacc()

    emt = nc.declare_dram_parameter("emt", [C, L * W], f32, isOutput=False)
    tagsr = nc.declare_dram_parameter("tagsr", [C, L * W], u8, isOutput=False)
    trans = nc.declare_dram_parameter("trans", [C, C], f32, isOutput=False)
    stend = nc.declare_dram_parameter("stend", [C, 2], f32, isOutput=False)
    out = nc.declare_dram_parameter("out", [BL], f32, isOutput=True)

    do_gold = not DBG_NO_GOLD
    full = DBG_NBLK == NBLK

    with tile.TileContext(nc) as tc, ExitStack() as ctx:
        singles = ctx.enter_context(tc.tile_pool(name="singles", bufs=1))
        empool = ctx.enter_context(tc.tile_pool(name="empool", bufs=4))
        embf_pool = ctx.enter_context(tc.tile_pool(name="embf", bufs=3))
        tgpool = ctx.enter_context(tc.tile_pool(name="tgpool", bufs=3))
        ohpool = ctx.enter_context(tc.tile_pool(name="ohpool", bufs=4))
        gupool = ctx.enter_context(tc.tile_pool(name="gupool", bufs=3))
        xpool = ctx.enter_context(tc.tile_pool(name="xpool", bufs=6))
        tmp_pool = ctx.enter_context(tc.tile_pool(name="tmp", bufs=5))
        s_psum = ctx.enter_context(tc.tile_pool(name="s_psum", bufs=3, space="PSUM"))
        u_psum = ctx.enter_context(tc.tile_pool(name="u_psum", bufs=2, space="PSUM"))
        c_psum = ctx.enter_context(tc.tile_pool(name="c_psum", bufs=1, space="PSUM"))
        g_psum = ctx.enter_context(tc.tile_pool(name="g_psum", bufs=1, space="PSUM"))
        r_psum = ctx.enter_context(tc.tile_pool(name="r_psum", bufs=1, space="PSUM"))

        # ---- constants -------------------------------------------------
        t_sb = singles.tile([C, C], f32)
        nc.sync.dma_start(out=t_sb, in_=trans[:, :])
        stend_sb = singles.tile([C, 2], f32)
        nc.sync.dma_start(out=stend_sb, in_=stend[:, :])
        start_sb = stend_sb[:, 0:1]
        end_sb = stend_sb[:, 1:2]

        # E (bf16) for the scan; T (bf16) for the gold transition lookup
        e_bf = singles.tile([C, C], bf16)
        nc.scalar.activation(e_bf, t_sb, Act.Exp)
        t_bf = singles.tile([C, C], bf16)
        nc.vector.tensor_copy(t_bf, t_sb)
        # u = exp(end)
        u_sb = singles.tile([C, 1], f32)
        nc.scalar.activation(u_sb, end_sb, Act.Exp)

        ident_bf = singles.tile([C, C], bf16)
        make_identity(nc, ident_bf)
        onescol_bf = singles.tile([C, 1], bf16)
        nc.vector.memset(onescol_bf, 1.0)
        ones128_bf = singles.tile([C, C], bf16)
        nc.vector.memset(ones128_bf, 1.0)

        # -C0 bias produced on ACT so the emission exp carries only the
        # DMA wait (big ISA structs have a single sync-wait slot)
        negc0_sb = singles.tile([C, 1], f32)
        nc.scalar.activation(negc0_sb, stend_sb[:, 0:1], Act.Copy, bias=-C0, scale=0.0)
        # early DVE read of stend_sb so later DVE ops that use start/end
        # slices carry no extra DMA-sem wait (some ISA structs allow only
        # one sync wait)
        stend_dve = singles.tile([C, 2], f32)
        nc.vector.tensor_copy(stend_dve, stend_sb)
        expstart_sb = singles.tile([C, 1], f32)
        nc.scalar.activation(expstart_sb, stend_sb[:, 0:1], Act.Exp)

        iota_i = singles.tile([C, 1], dt.int32)
        nc.gpsimd.iota(iota_i, pattern=[[0, 1]], base=0, channel_multiplier=1)
        iota_f = singles.tile([C, 1], f32)
        nc.vector.tensor_copy(iota_f, iota_i)

        # resident exp(emit - C0) for the whole sequence (bf16, 64KB/partition)
        ee_all = singles.tile([C, L * W], bf16)
        # log-offset accumulator (renorm logs from both passes), [1, W]
        o_acc = singles.tile([1, W], f32)
        nc.vector.memset(o_acc, 0.0)
        f1_sb = singles.tile([C, W], bf16)
        oh_zero = singles.tile([C, W], bf16)
        nc.vector.memset(oh_zero, 0.0)
        gold_sb = singles.tile([1, WB], f32)
        nc.vector.memset(gold_sb, 0.0)

        gold_acc = g_psum.tile([1, WB], f32, tag="gold_acc", name="gold_acc") if (do_gold and DBG_GOLD_LVL >= 3) else None

        # PE warm-up matmuls: absorb the ACT tick, then the GPSIMD tick, so
        # every later matmul waits on at most one new proc (big ISA structs
        # have a single sync-wait slot)
        warm_ps = s_psum.tile([1, 4], f32, tag="s", name="warm")
        nc.tensor.matmul(warm_ps, lhsT=e_bf[:, 0:1], rhs=e_bf[:, 0:4], start=True, stop=True)
        warm2_ps = s_psum.tile([1, 4], f32, tag="s", name="warm2")
        nc.tensor.matmul(warm2_ps, lhsT=ident_bf[:, 0:1], rhs=ident_bf[:, 0:4], start=True, stop=True)

        # ---------------------------------------------------------------
        def renorm(x_new, track, h):
            crep = c_psum.tile([C, W2], f32)
            nc.tensor.matmul(crep, lhsT=ones128_bf, rhs=x_new, start=True, stop=True)
            rec = tmp_pool.tile([C, W2], f32, tag="rec")
            nc.vector.reciprocal(rec, crep)
            x_n = xpool.tile([C, W2], bf16, tag=f"X{h}", name=f"xn{h}")
            nc.vector.tensor_tensor(x_n, x_new, rec, op=Alu.mult)
            if track:
                lg = tmp_pool.tile([1, W2], f32, tag="lg")
                nc.scalar.activation(lg, crep[0:1, :], Act.Ln)
                nc.vector.tensor_tensor(
                    o_acc[0:1, h * W2 : (h + 1) * W2],
                    o_acc[0:1, h * W2 : (h + 1) * W2], lg, op=Alu.add,
                )
            return x_n

        def scan_step_half(x, t, h, is_pass1):
            # half h covers columns [h*W2, (h+1)*W2) of the chunk x batch grid
            s_ps = s_psum.tile([C, W2], f32, tag="s", name=f"s{h}")
            nc.tensor.matmul(s_ps, lhsT=e_bf, rhs=x, start=True, stop=True)
            x_new = xpool.tile([C, W2], bf16, tag=f"X{h}", name=f"x{h}")
            base = t * W + h * W2
            nc.vector.tensor_tensor(
                x_new, s_ps, ee_all[:, base : base + W2], op=Alu.mult
            )
            if t == 0 and h == 0:
                # true chunk-0 init: exp(start + emit_0 - C0) == ee cols
                nc.vector.tensor_copy(x_new[:, 0:BL], ee_all[:, 0:BL])
            x = x_new
            if (t + 1) % K_RENORM == 0 and not DBG_NO_RENORM:
                if is_pass1:
                    if t != L - 1:
                        x = renorm(x, track=(t >= F), h=h)
                else:
                    x = renorm(x, track=True, h=h)
            return x

        # ---- pass 1 (gold interleaved) --------------------------------
        xa = xpool.tile([C, W2], bf16, tag="X0", name="xa0")
        nc.vector.memset(xa, 1.0)
        xb = xpool.tile([C, W2], bf16, tag="X1", name="xb0")
        nc.vector.memset(xb, 1.0)

        oh_prev_half = oh_zero[:, :]        # one-hot of previous macro-step
        keep_tg0 = None
        last_oh = None

        for blk in range(DBG_NBLK):
            t0 = TB * blk
            col0 = t0 * W

            em_t = empool.tile([C, WB], f32, tag="em")
            nc.sync.dma_start(out=em_t, in_=emt[:, col0 : col0 + WB])
            # resident scaled-exp emissions
            nc.scalar.activation(
                ee_all[:, col0 : col0 + WB], em_t, Act.Exp, bias=negc0_sb
            )
            if blk == 0:
                # chunk-0 init cols get exp(start + emit_0 - C0): scale the
                # just-computed exp(emit-C0) by exp(start); all operands are
                # ACT-local so this op needs no cross-engine sync wait
                nc.scalar.activation(
                    ee_all[:, 0:BL], ee_all[:, 0:BL], Act.Copy,
                    scale=expstart_sb,
                )

            if do_gold:
                # bf16 emissions for the gold PSUM accumulation
                em_bf = embf_pool.tile([C, WB], bf16, tag="embf")
                nc.vector.tensor_copy(em_bf, em_t)
                # gold: tags one-hot for this block (built on the otherwise
                # idle GPSIMD engine)
                tgrep = tgpool.tile([C, WB], u8, tag="tg")
                nc.sync.dma_start(out=tgrep, in_=tagsr[:, col0 : col0 + WB])
                oh = ohpool.tile([C, WB], bf16, tag="oh")
                nc.gpsimd.tensor_scalar(oh, tgrep, iota_f, None, op0=Alu.is_equal)
                if blk == 0:
                    keep_tg0 = tgpool.tile([C, W], u8, tag="tg0")
                    nc.gpsimd.tensor_copy(keep_tg0, tgrep[:, 0:W])

                if DBG_GOLD_LVL >= 2:
                    # U = T^T @ onehot_prev (+ emit)
                    u_ps = u_psum.tile([C, WB], f32, tag="U")
                    nc.tensor.matmul(
                        u_ps, lhsT=t_bf, rhs=oh_prev_half,
                        start=True, stop=False, skip_group_check=True,
                    )
                    nc.tensor.matmul(
                        u_ps, lhsT=ident_bf, rhs=em_bf, start=False, stop=True,
                        skip_group_check=True,
                    )
                    # drain U+em to SBUF on ACT, select on GPSIMD: keeps the
                    # serial-scan engine (DVE) free
                    usel_sb = gupool.tile([C, WB], bf16, tag="usb")
                    nc.scalar.activation(usel_sb, u_ps, Act.Copy)
                    gu = gupool.tile([C, WB], bf16, tag="gu")
                    nc.gpsimd.tensor_tensor(gu, oh, usel_sb, op=Alu.mult)
                    if DBG_GOLD_LVL >= 3:
                        seg_first = blk % GOLD_SEG == 0
                        seg_last = blk % GOLD_SEG == GOLD_SEG - 1
                        nc.tensor.matmul(
                            gold_acc, lhsT=onescol_bf, rhs=gu, start=seg_first,
                            stop=seg_last, skip_group_check=True,
                        )
                        if seg_last:
                            nc.vector.tensor_tensor(
                                gold_sb, gold_sb, gold_acc, op=Alu.add
                            )
                oh_prev_half = oh[:, :]
                last_oh = oh

            # scan macro-steps for this block
            for ts in range(TB):
                xa = scan_step_half(xa, t0 + ts, 0, is_pass1=True)
                xb = scan_step_half(xb, t0 + ts, 1, is_pass1=True)

        nc.vector.tensor_copy(f1_sb[:, 0:W2], xa)
        nc.vector.tensor_copy(f1_sb[:, W2:W], xb)

        if do_gold and full and DBG_GOLD_LVL >= 3:
            # deferred chunk-boundary gold terms: T[tag(L-1,c-1), tag(0,c)]
            u0_ps = u_psum.tile([C, (NCH - 1) * BL], f32, tag="U")
            nc.tensor.matmul(
                u0_ps, lhsT=t_bf, rhs=last_oh[:, 0 : (NCH - 1) * BL],
                start=True, stop=True, skip_group_check=True,
            )
            oh0 = ohpool.tile([C, (NCH - 1) * BL], bf16, tag="oh0")
            nc.vector.tensor_scalar(
                oh0, keep_tg0[:, BL:W], iota_f, None, op0=Alu.is_equal
            )
            gu0 = gupool.tile([C, (NCH - 1) * BL], bf16, tag="gu")
            nc.vector.tensor_tensor(gu0, oh0, u0_ps, op=Alu.mult)
            nc.tensor.matmul(
                gold_acc[0:1, BL:W], lhsT=onescol_bf, rhs=gu0,
                start=True, stop=False, skip_group_check=True,
            )
            # start_transitions term: onehot(tag_0, chunk 0) * start
            oh_s = ohpool.tile([C, BL], bf16, tag="oh0")
            nc.vector.tensor_scalar(
                oh_s, keep_tg0[:, 0:BL], iota_f, None, op0=Alu.is_equal
            )
            gstart = gupool.tile([C, BL], bf16, tag="gu")
            nc.vector.tensor_scalar(
                gstart, oh_s, stend_dve[:, 0:1], None, op0=Alu.mult
            )
            nc.tensor.matmul(
                gold_acc[0:1, 0:BL], lhsT=onescol_bf, rhs=gstart,
                start=False, stop=False, skip_group_check=True,
            )
            # end_transitions term: onehot(tag_last) * end
            gend = gupool.tile([C, BL], bf16, tag="gu")
            nc.vector.tensor_scalar(
                gend, last_oh[:, WB - BL : WB], stend_dve[:, 1:2], None, op0=Alu.mult
            )
            nc.tensor.matmul(
                gold_acc[0:1, WB - BL : WB], lhsT=onescol_bf, rhs=gend,
                start=False, stop=True, skip_group_check=True,
            )
            # the deferred segment's first matmul used start=True, zeroing the
            # whole bank, and every column is written exactly once -> one add
            nc.vector.tensor_tensor(
                gold_sb[0:1, 0:WB], gold_sb[0:1, 0:WB], gold_acc[0:1, 0:WB],
                op=Alu.add,
            )

        # ---- pass 2: 32-step head fixup -------------------------------
        if not DBG_NO_PASS2:
            x2a = xpool.tile([C, W2], bf16, tag="X0", name="x2a")
            nc.vector.memset(x2a, 1.0)
            nc.vector.tensor_copy(x2a[:, BL:W2], f1_sb[:, 0 : W2 - BL])
            x2b = xpool.tile([C, W2], bf16, tag="X1", name="x2b")
            nc.vector.tensor_copy(x2b, f1_sb[:, W2 - BL : W - BL])
            xa, xb = x2a, x2b
            for t in range(F):
                xa = scan_step_half(xa, t, 0, is_pass1=False)
                xb = scan_step_half(xb, t, 1, is_pass1=False)

        # ---- final assembly -------------------------------------------
        # logZ_b = sum_c o_acc + 2048*C0 + log(u^T F1[last chunk])
        v16 = tmp_pool.tile([C, BL], bf16, tag="v16")
        nc.vector.tensor_scalar(
            v16, f1_sb[:, W - BL : W], u_sb, None, op0=Alu.mult
        )
        ud_ps = r_psum.tile([1, BL], f32)
        nc.tensor.matmul(ud_ps, lhsT=onescol_bf, rhs=v16, start=True, stop=True)
        logud = tmp_pool.tile([1, BL], f32, tag="asm")
        nc.scalar.activation(logud, ud_ps, Act.Ln)

        # sum o_acc over chunks: view [1, (c b)] -> [1, b, c], reduce inner
        oz = tmp_pool.tile([1, BL], f32, tag="asm")
        nc.vector.tensor_reduce(
            oz,
            o_acc.rearrange("p (c b) -> p b c", c=NCH, b=BL),
            axis=mybir.AxisListType.X,
            op=Alu.add,
        )
        loss = tmp_pool.tile([1, BL], f32, tag="asm")
        if do_gold and full and DBG_GOLD_LVL >= 3:
            # gold: view [1, (ts c b)] -> [1, b, (ts c)], reduce inner
            gr = tmp_pool.tile([1, BL], f32, tag="asm")
            nc.vector.tensor_reduce(
                gr,
                gold_sb.rearrange("p (t c b) -> p b (t c)", t=TB, c=NCH, b=BL),
                axis=mybir.AxisListType.X,
                op=Alu.add,
            )
            nc.vector.tensor_tensor(loss, gr, oz, op=Alu.subtract)
        else:
            nc.vector.tensor_scalar(loss, oz, -1.0, None, op0=Alu.mult)
        nc.vector.tensor_tensor(loss, loss, logud, op=Alu.subtract)
        nc.vector.tensor_scalar(loss, loss, -float(S) * C0, None, op0=Alu.add)
        nc.sync.dma_start(out=out[:], in_=loss[0:1, :])

    nc.finalize()
    return nc


_PROGRAM = None


def _get_program():
    global _PROGRAM
    if _PROGRAM is None:
        _PROGRAM = build_program()
    return _PROGRAM


def make_in_maps(emissions, transitions, start_transitions, end_transitions, tags):
    emissions = np.asarray(emissions, np.float32)
    transitions = np.asarray(transitions, np.float32)
    start_transitions = np.asarray(start_transitions, np.float32)
    end_transitions = np.asarray(end_transitions, np.float32)
    tags = np.asarray(tags)

    stend = np.ascontiguousarray(
        np.stack([start_transitions, end_transitions], axis=1)
    ).astype(np.float32)

    in_maps = []
    for k in range(NCORES):
        sl = slice(k * BL, (k + 1) * BL)
        # [BL, S, C] -> [C, S, BL] -> [C, NCH, L, BL] -> [C, L, NCH, BL]
        em = emissions[sl].transpose(2, 1, 0).reshape(C, NCH, L, BL)
        em = np.ascontiguousarray(em.transpose(0, 2, 1, 3)).reshape(C, L * W)
        # tags -> [L, NCH*BL] u8, replicated across 128 partitions
        tg = tags[sl].T.reshape(NCH, L, BL).transpose(1, 0, 2).reshape(L * W)
        tg = np.ascontiguousarray(
            np.broadcast_to(tg.astype(np.uint8)[None, :], (C, L * W))
        )
        in_maps.append(
            {"emt": em, "tagsr": tg, "trans": transitions, "stend": stend}
        )
    return in_maps


def kernel(emissions, transitions, start_transitions, end_transitions, tags, mask):
    from concourse.bass_utils import run_bass_kernel_spmd

    nc = _get_program()
    in_maps = make_in_maps(
        emissions, transitions, start_transitions, end_transitions, tags
    )
    res = run_bass_kernel_spmd(nc, in_maps, list(range(NCORES))).results
    parts = np.concatenate([np.asarray(r["out"], np.float32) for r in res])
    return np.float32(-parts.mean())
